# revision 1
# baseline (speedup 1.0000x reference)
import sys

if "/opt/trn_rl_repo" not in sys.path:
    sys.path.insert(0, "/opt/trn_rl_repo")

from contextlib import ExitStack

import numpy as np

from concourse import bacc, masks, mybir, tile
from concourse.bass_utils import run_bass_kernel_spmd

f32 = mybir.dt.float32
bf16 = mybir.dt.bfloat16
Alu = mybir.AluOpType
Act = mybir.ActivationFunctionType

P = 128
C_MAGIC = 1.5 * 2 ** 23
QDIV = 127.5 * (1.0 - 2.0 ** -20)
INV_QDIV = 1.0 / QDIV
TINY = 1e-30

M, K, N = 8192, 4096, 4096
MG, NG = 2, 4
M_loc, N_loc = M // MG, N // NG
N_CORES = MG * NG

XBAR_KT = 24
HEAD_MIS = 3


def build_aqt(nc, M_loc, K, N_loc, W=512):
    KT, MT = K // P, M_loc // P
    NB = N_loc // W
    NT = N_loc // P
    TPB = NT // NB
    H = K // 2
    HT = H // P
    assert (KT - XBAR_KT) % 8 == 0

    lhs = nc.declare_dram_parameter("lhs", [M_loc, K], f32, isOutput=False)
    rhsT = nc.declare_dram_parameter("rhsT", [N_loc, K], f32, isOutput=False)
    out = nc.declare_dram_parameter("out", [M_loc, N_loc], f32, isOutput=True)

    with tile.TileContext(nc) as tc, ExitStack() as ctx:
        pool = lambda name, bufs: ctx.enter_context(tc.tile_pool(name=name, bufs=bufs))
        const_pool = pool("constp", 1)
        qr_pool = pool("qr", 1)
        raws = pool("raws", 4)
        rt1 = pool("rt1", 1)
        rqf = pool("rqf", 2)
        rsc = pool("rsc", 4)
        lt1 = pool("lt1", 1)
        lqb = pool("lqb", 2)
        lqt = pool("lqt", 3)
        lsc = pool("lsc", 1)
        sml = pool("sml", 6)
        opool = pool("o1", 2)
        psum = ctx.enter_context(tc.tile_pool(name="psum", bufs=3, space="PSUM"))
        psumT = ctx.enter_context(tc.tile_pool(name="psumT", bufs=2, space="PSUM"))

        ident = const_pool.tile([P, P], bf16)
        masks.make_identity(nc, ident[:])

        s_l_all = lsc.tile([P, MT], f32)
        qr_nb = [qr_pool.tile([P, KT, W], bf16, name=f"qrnb{nb}")
                 for nb in range(NB)]

        rraw_t, rsc_t = {}, {}

        def emit_rhs_load(j):
            raw = raws.tile([P, K], f32, name="raw")
            nc.sync.dma_start(raw[:], rhsT[j * P:(j + 1) * P, :])
            am = sml.tile([P, 1], f32, name="ram")
            nc.vector.tensor_reduce(am[:], raw[:], axis=mybir.AxisListType.X,
                                    op=Alu.max, apply_absolute_value=True)
            s_col = rsc.tile([P, 1], f32, name="rs")
            nc.vector.tensor_scalar(s_col[:], am[:], TINY, INV_QDIV,
                                    op0=Alu.max, op1=Alu.mult)
            r_col = sml.tile([P, 1], f32, name="rr")
            nc.vector.reciprocal(r_col[:], s_col[:])
            rraw_t[j] = raw
            rsc_t[j] = (s_col, r_col)

        def emit_rhs_quant(j):
            nb, jo = divmod(j, TPB)
            raw = rraw_t.pop(j)
            s_col, r_col = rsc_t.pop(j)
            for h in range(2):
                t1 = rt1.tile([P, H], f32, name="rt1")
                nc.scalar.activation(t1[:], raw[:, h * H:(h + 1) * H], Act.Copy,
                                     bias=C_MAGIC, scale=r_col[:])
                qf = rqf.tile([P, H], bf16, name="rqf")
                nc.vector.tensor_scalar(qf[:], t1[:], C_MAGIC, s_col[:],
                                        op0=Alu.subtract, op1=Alu.mult)
                nc.sync.dma_start_transpose(
                    qr_nb[nb][:, h * HT:(h + 1) * HT, jo * P:(jo + 1) * P], qf[:])

        lraw_t, lam_t, lqb_t, lqt_t = {}, {}, {}, {}

        def emit_lhs_load(mi):
            rs = slice(mi * P, (mi + 1) * P)
            raw = raws.tile([P, K], f32, name="raw")
            nc.sync.dma_start(raw[:], lhs[rs, :])
            am = sml.tile([P, 1], f32, name="lam")
            nc.vector.tensor_reduce(am[:], raw[:], axis=mybir.AxisListType.X,
                                    op=Alu.max, apply_absolute_value=True)
            lraw_t[mi] = raw
            lam_t[mi] = am

        def emit_lhs_quant(mi):
            raw, am = lraw_t.pop(mi), lam_t.pop(mi)
            s_col = s_l_all[:, mi:mi + 1]
            nc.vector.tensor_scalar(s_col, am[:], TINY, INV_QDIV,
                                    op0=Alu.max, op1=Alu.mult)
            r_col = sml.tile([P, 1], f32, name="lr")
            nc.vector.reciprocal(r_col[:], s_col)
            t1 = lt1.tile([P, K], f32, name="lt1")
            nc.scalar.activation(t1[:], raw[:], Act.Copy,
                                 bias=C_MAGIC, scale=r_col[:])
            qb = lqb.tile([P, K], bf16, name="lqb")
            nc.scalar.activation(qb[:], t1[:], Act.Copy, bias=-C_MAGIC)
            lqb_t[mi] = qb

        def emit_lhs_transpose(mi, xbar_kt):
            qb = lqb_t.pop(mi)
            qt = lqt.tile([P, KT, P], bf16, name="lqt")
            if xbar_kt > 0:
                nc.sync.dma_start_transpose(qt[:, 0:xbar_kt, :],
                                            qb[:, 0:xbar_kt * P])
            for g in range((KT - xbar_kt) // 8):
                pt = psumT.tile([P, 8 * P], bf16, name="pt")
                for t in range(8):
                    kt = xbar_kt + g * 8 + t
                    nc.tensor.transpose(pt[:, t * P:(t + 1) * P],
                                        qb[:, kt * P:(kt + 1) * P],
                                        ident[:])
                nc.vector.tensor_copy(qt[:, xbar_kt + g * 8:xbar_kt + (g + 1) * 8, :],
                                      pt[:])
            lqt_t[mi] = qt

        def emit_mm_group(mi, nb, last):
            qt = lqt_t[mi]
            if last:
                del lqt_t[mi]
            rs = slice(mi * P, (mi + 1) * P)
            ps = psum.tile([P, W], f32, name="ps")
            for kt in range(KT):
                nc.tensor.matmul(ps[:], qt[:, kt, :], qr_nb[nb][:, kt, :],
                                 start=(kt == 0), stop=(kt == KT - 1))
            o1 = opool.tile([P, W], f32, name="o1")
            nc.scalar.activation(o1[:], ps[:], Act.Copy, bias=0.0,
                                 scale=s_l_all[:, mi:mi + 1])
            nc.sync.dma_start(out[rs, nb * W:(nb + 1) * W], o1[:])

        emit_lhs_load(0)
        emit_rhs_load(0)
        emit_rhs_load(1)
        emit_rhs_load(2)
        emit_lhs_quant(0)
        emit_lhs_transpose(0, 0)
        emit_lhs_load(1)
        emit_rhs_quant(0)
        emit_rhs_load(3)
        emit_rhs_quant(1)
        emit_rhs_load(4)
        emit_rhs_quant(2)
        emit_rhs_load(5)
        emit_lhs_quant(1)
        emit_lhs_transpose(1, 0)
        emit_lhs_load(2)
        emit_rhs_quant(3)
        emit_rhs_load(6)
        emit_rhs_quant(4)
        emit_rhs_load(7)
        emit_rhs_quant(5)
        emit_lhs_load(3)
        emit_lhs_quant(2)
        emit_lhs_transpose(2, 0)
        emit_rhs_quant(6)
        emit_rhs_quant(7)
        emit_lhs_quant(3)
        emit_lhs_transpose(3, XBAR_KT)
        emit_lhs_load(4)
        emit_lhs_load(5)
        for mi in range(HEAD_MIS):
            emit_mm_group(mi, 0, last=False)
        for mi in range(HEAD_MIS):
            emit_mm_group(mi, 1, last=True)
        emit_lhs_quant(4)
        emit_lhs_transpose(4, XBAR_KT)
        emit_lhs_load(6)

        for mi in range(HEAD_MIS, MT):
            if mi >= HEAD_MIS + 1:
                if mi + 3 < MT:
                    emit_lhs_load(mi + 3)
                if mi + 1 < MT and mi + 1 > HEAD_MIS + 1:
                    emit_lhs_quant(mi + 1)
                    emit_lhs_transpose(mi + 1, XBAR_KT)
            for nb in range(NB):
                emit_mm_group(mi, nb, last=(nb == NB - 1))
    return nc


_COMPILED_NC = None


def _get_compiled():
    global _COMPILED_NC
    if _COMPILED_NC is None:
        nc = bacc.Bacc("TRN2", target_bir_lowering=False, debug=False,
                       num_devices=N_CORES)
        build_aqt(nc, M_loc, K, N_loc)
        nc.compile()
        _COMPILED_NC = nc
    return _COMPILED_NC


def _shard(lhs, rhs):
    rhsT = np.ascontiguousarray(rhs.T)
    in_maps = []
    for i in range(N_CORES):
        mg, ng = divmod(i, NG)
        in_maps.append({
            "lhs": np.ascontiguousarray(lhs[mg * M_loc:(mg + 1) * M_loc, :]),
            "rhsT": rhsT[ng * N_loc:(ng + 1) * N_loc, :],
        })
    return in_maps


def kernel(lhs, rhs, _trace=False, _trace_kwargs=None):
    lhs = np.asarray(lhs, np.float32)
    rhs = np.asarray(rhs, np.float32)
    nc = _get_compiled()
    res = run_bass_kernel_spmd(nc, _shard(lhs, rhs), core_ids=list(range(N_CORES)),
                               trace=_trace, **(_trace_kwargs or {}))
    out = np.empty((M, N), np.float32)
    for i in range(N_CORES):
        mg, ng = divmod(i, NG)
        out[mg * M_loc:(mg + 1) * M_loc, ng * N_loc:(ng + 1) * N_loc] = \
            res.results[i]["out"]
    kernel.last_result = res
    return out

